# revision 2
# baseline (speedup 1.0000x reference)
"""MinGRU kernel for Trainium2 (8 NeuronCores, Bass/Tile).

Reference computation (B=4, L=8192, D=512, fp32):
    gates = sigmoid(x @ Wg.T + bg)
    cands = tanh(x @ Wc.T + bc)
    h_t   = (1 - g_t) * h_{t-1} + g_t * c_t   (scan along L, h_0 = 0)

Sharding: core c -> (batch b = c//2, channel half = c%2). Each core computes
its batch's full L range for 256 of the 512 output channels; the scan along L
is per (b, channel) so no cross-core communication is needed.

Layout: host pre-transposes x[b] to [D, L] and weights to [D, 256] (lhsT),
both in fp16, so every device DMA is fully contiguous and the x stream is half
the fp32 bytes (the fp32 feed saturates the ~358 GB/s HBM read port; fp16
leaves headroom so the PE is never starved). On device, matmuls keep channels
on partitions and tokens on the free axis, which is exactly the layout
tensor_tensor_scan needs (recurrence runs along the free dim). The scan uses
    state = (a * state) - bneg,   a = sigmoid(-z_g - bg) = 1 - g,
    bneg = (a - 1) * c = -g * c
so a single scalar_tensor_tensor op feeds the scan.

Both 128-channel e-tiles of a core live in one [128, 2, seg] tile per operand
(one activation covers two PSUM banks, one h DMA covers both e-tiles),
roughly halving instruction and tile-buffer count vs per-e-tile tiles.
Output is [256, L] per core; the host reassembles [B, L, D].

Precision: x and W feed the PE as fp16 (1 row/cycle at any moving size);
PSUM accumulation is fp32. The activation outputs a/c, the scan operands, and
the stored h are fp16 (the scan's internal state stays fp32 per the ISA); h
is written to HBM as fp16 and upcast on the host, halving output DMA bytes.
End-to-end max relative error ~2e-3.
"""

import os
import sys

sys.path.insert(0, "/opt/trn_rl_repo")

import numpy as np

import concourse.bacc as bacc
import concourse.bass as bass
import concourse.mybir as mybir
from concourse.bass_utils import run_bass_kernel_spmd
from concourse.tile import TileContext

B, L, D = 4, 8192, 512
NCORES = 8
EH = D // 2          # output channels per core
NET = EH // 128      # e-tiles per core (2)
NDC = D // 128       # contraction chunks (4)
NSUB = 512           # matmul moving free dim (= 1 fp32 PSUM bank)
# Token segments: small head segments start the PE/scan pipeline early, small
# tail segments shrink the post-last-DMA drain (act -> bneg -> scan -> h DMA
# chain on the final tokens).
SEGS = [256, 256, 512, 1024, 1024, 1024, 1024, 1024, 1024, 512, 256, 128, 128]
assert sum(SEGS) == L
MAXSEG = max(SEGS)

FP32 = mybir.dt.float32
F16 = mybir.dt.float16
_last_results = None


def build_nc() -> bass.Bass:
    # Bacc (not plain Bass): its compile() runs move_matmul_waits_to_ldweights
    # and generate_event_semaphores, which split multi-sem waits to satisfy the
    # TRN2 per-instruction wait-slot limits walrus enforces.
    nc = bacc.Bacc()

    xT = nc.dram_tensor("xT", [D, L], F16, kind="ExternalInput")
    wgT = nc.dram_tensor("wgT", [D, EH], F16, kind="ExternalInput")
    wcT = nc.dram_tensor("wcT", [D, EH], F16, kind="ExternalInput")
    # biases packed [128, 4]: cols 0..1 = bg per e-tile, 2..3 = bc per e-tile
    bias = nc.dram_tensor("bias", [128, 2 * NET], FP32, kind="ExternalInput")
    h = nc.dram_tensor("h", [EH, L], F16, kind="ExternalOutput")

    op = mybir.AluOpType
    act = mybir.ActivationFunctionType

    with TileContext(nc) as tc:
        with (
            tc.tile_pool(name="consts", bufs=1) as consts,
            tc.tile_pool(name="xpool", bufs=4) as xpool,
            tc.tile_pool(name="work", bufs=2) as work,
            tc.tile_pool(name="hpool", bufs=2) as hpool,
            tc.tile_pool(name="psum", bufs=1, space="PSUM") as psum,
        ):
            # Sync HWDGE queue order: wg -> x seg 0 -> wc -> x seg 1 -> ...
            # The first matmul group only needs wg + the first x segment, so
            # this starts the PE as early as possible. Biases ride the SWDGE
            # (gpsimd) queue.
            wg_sb = consts.tile([128, NDC, EH], F16)
            wc_sb = consts.tile([128, NDC, EH], F16)
            nc.sync.dma_start(wg_sb, wgT.rearrange("(c p) e -> p c e", p=128))
            x0_sb = xpool.tile([128, NDC, MAXSEG], F16, tag="x", name="x_0")[
                :, :, : SEGS[0]
            ]
            nc.sync.dma_start(x0_sb, xT[:, 0 : SEGS[0]].rearrange("(c p) l -> p c l", p=128))
            nc.sync.dma_start(wc_sb, wcT.rearrange("(c p) e -> p c e", p=128))

            bias_sb = consts.tile([128, 2 * NET], FP32)
            bgn_sb = consts.tile([128, NET], FP32)
            nc.gpsimd.dma_start(bias_sb, bias[:])
            nc.scalar.mul(bgn_sb, bias_sb[:, 0:NET], -1.0)
            bc_sb = bias_sb[:, NET : 2 * NET]

            carry = [None] * NET  # [128, 1] AP of the previous h column

            l0 = 0
            for t, lt in enumerate(SEGS):
                nbk = (lt + NSUB - 1) // NSUB  # PSUM banks this segment
                if t == 0:
                    x_sb = x0_sb
                else:
                    x_sb = xpool.tile([128, NDC, MAXSEG], F16, tag="x", name=f"x_{t}")[
                        :, :, :lt
                    ]
                    nc.sync.dma_start(
                        x_sb, xT[:, l0 : l0 + lt].rearrange("(c p) l -> p c l", p=128)
                    )
                a_t = work.tile([128, NET, MAXSEG], F16, tag="a", name=f"a_{t}")
                c_t = work.tile([128, NET, MAXSEG], F16, tag="c", name=f"c_{t}")
                bn_t = work.tile([128, NET, MAXSEG], F16, tag="b", name=f"b_{t}")
                h_t = hpool.tile([128, NET, MAXSEG], F16, tag="h", name=f"h_{t}")
                for et in range(NET):
                    esl = slice(et * 128, (et + 1) * 128)
                    # One 4-bank PSUM tile per e-tile: [*, 0, b, :] = z_g
                    # banks, [*, 1, b, :] = z_c banks.
                    pz = psum.tile(
                        [128, 2, 2, NSUB], FP32, tag=f"pz{et}", name=f"pz{et}_{t}"
                    )
                    for n in range(nbk):
                        w = min(NSUB, lt - n * NSUB)
                        nsl = slice(n * NSUB, n * NSUB + w)
                        for proj in range(2):
                            wsrc = wg_sb if proj == 0 else wc_sb
                            for dc in range(NDC):
                                nc.tensor.matmul(
                                    pz[:, proj, n, :w],
                                    wsrc[:, dc, esl],
                                    x_sb[:, dc, nsl],
                                    start=(dc == 0),
                                    stop=(dc == NDC - 1),
                                )
                    # a = sigmoid(-(z_g + bg)) = 1 - g ; c = tanh(z_c + bc)
                    # One activation per projection spans both PSUM banks.
                    wa = min(NSUB, lt)
                    nc.scalar.activation(
                        a_t[:, et, :lt].rearrange("p (b n) -> p b n", b=nbk),
                        pz[:, 0, :nbk, :wa],
                        act.Sigmoid,
                        bias=bgn_sb[:, et : et + 1],
                        scale=-1.0,
                    )
                    nc.scalar.activation(
                        c_t[:, et, :lt].rearrange("p (b n) -> p b n", b=nbk),
                        pz[:, 1, :nbk, :wa],
                        act.Tanh,
                        bias=bc_sb[:, et : et + 1],
                        scale=1.0,
                    )
                    # bneg = (a - 1) * c = -g * c  (one DVE op, full segment)
                    nc.vector.scalar_tensor_tensor(
                        bn_t[:, et, :lt], a_t[:, et, :lt], 1.0, c_t[:, et, :lt],
                        op.subtract, op.mult,
                    )
                    # h = a * h_prev - bneg  (fp32 state in HW, fp16 storage)
                    init = 0.0 if carry[et] is None else carry[et]
                    nc.vector.tensor_tensor_scan(
                        h_t[:, et, :lt], a_t[:, et, :lt], bn_t[:, et, :lt],
                        init, op.mult, op.subtract,
                    )
                    carry[et] = h_t[:, et, lt - 1 : lt]
                # h writes on the SWDGE queue: keeps the sync HWDGE queue a
                # pure x-feed. One DMA covers both e-tiles.
                nc.gpsimd.dma_start(
                    h[:, l0 : l0 + lt].rearrange("(e p) l -> p e l", p=128),
                    h_t[:, :, :lt],
                )
                l0 += lt
    return nc


def _in_maps(x, Wg, bg, Wc, bc):
    maps = []
    xT = {}
    for c in range(NCORES):
        b, eh = c // 2, c % 2
        e0 = eh * EH
        if b not in xT:
            xT[b] = np.ascontiguousarray(x[b].T.astype(np.float16))
        bias_pack = np.concatenate(
            [
                bg[e0 : e0 + EH].reshape(NET, 128).T,
                bc[e0 : e0 + EH].reshape(NET, 128).T,
            ],
            axis=1,
        )
        maps.append(
            {
                "xT": xT[b],
                "wgT": np.ascontiguousarray(Wg[e0 : e0 + EH].T.astype(np.float16)),
                "wcT": np.ascontiguousarray(Wc[e0 : e0 + EH].T.astype(np.float16)),
                "bias": np.ascontiguousarray(bias_pack.astype(np.float32)),
            }
        )
    return maps


def kernel(x, Wg, bg, Wc, bc):
    global _last_results
    x = np.asarray(x, dtype=np.float32)
    Wg = np.asarray(Wg, dtype=np.float32)
    bg = np.asarray(bg, dtype=np.float32)
    Wc = np.asarray(Wc, dtype=np.float32)
    bc = np.asarray(bc, dtype=np.float32)

    nc = build_nc()
    if not nc.is_finalized():
        nc.finalize()
    res = run_bass_kernel_spmd(
        nc,
        _in_maps(x, Wg, bg, Wc, bc),
        list(range(NCORES)),
        tmpdir=os.environ.get("KERNEL_TMPDIR"),
    )
    _last_results = res

    out = np.empty((B, L, D), dtype=np.float32)
    for b in range(B):
        hb = np.concatenate(
            [res.results[2 * b]["h"], res.results[2 * b + 1]["h"]], axis=0
        ).astype(np.float32)
        out[b] = hb.T
    return out


# revision 5
# speedup vs baseline: 1.0018x; 1.0018x over previous
"""MinGRU kernel for Trainium2 (8 NeuronCores, Bass/Tile).

Reference computation (B=4, L=8192, D=512, fp32):
    gates = sigmoid(x @ Wg.T + bg)
    cands = tanh(x @ Wc.T + bc)
    h_t   = (1 - g_t) * h_{t-1} + g_t * c_t   (scan along L, h_0 = 0)

Sharding: core c -> (batch b = c//2, channel half = c%2). Each core computes
its batch's full L range for 256 of the 512 output channels; the scan along L
is per (b, channel) so no cross-core communication is needed.

Layout: host pre-transposes x[b] to [D, L] and weights to [D, 256] (lhsT),
both in fp16, so every device DMA is fully contiguous and the x stream is half
the fp32 bytes (the fp32 feed saturates the ~358 GB/s HBM read port; fp16
leaves headroom so the PE is never starved). On device, matmuls keep channels
on partitions and tokens on the free axis, which is exactly the layout
tensor_tensor_scan needs (recurrence runs along the free dim). The scan uses
    state = (a * state) - bneg,   a = sigmoid(-z_g - bg) = 1 - g,
    bneg = (a - 1) * c = -g * c
so a single scalar_tensor_tensor op feeds the scan.

Both 128-channel e-tiles of a core live in one [128, 2, seg] tile per operand
(one activation covers two PSUM banks, one h DMA covers both e-tiles),
roughly halving instruction and tile-buffer count vs per-e-tile tiles.
Output is [256, L] per core; the host reassembles [B, L, D].

Precision: x and W feed the PE as fp16 (1 row/cycle at any moving size);
PSUM accumulation is fp32. The activation outputs a/c, the scan operands, and
the stored h are fp16 (the scan's internal state stays fp32 per the ISA); h
is written to HBM as fp16 and upcast on the host, halving output DMA bytes.
End-to-end max relative error ~2e-3.
"""

import os
import sys

sys.path.insert(0, "/opt/trn_rl_repo")

import numpy as np

import concourse.bacc as bacc
import concourse.bass as bass
import concourse.mybir as mybir
from concourse.bass_utils import run_bass_kernel_spmd
from concourse.tile import TileContext

B, L, D = 4, 8192, 512
NCORES = 8
EH = D // 2          # output channels per core
NET = EH // 128      # e-tiles per core (2)
NDC = D // 128       # contraction chunks (4)
NSUB = 512           # matmul moving free dim (= 1 fp32 PSUM bank)
# Token segments: small head segments start the PE/scan pipeline early, small
# tail segments shrink the post-last-DMA drain (act -> bneg -> scan -> h DMA
# chain on the final tokens).
SEGS = [256, 256, 512, 1024, 1024, 1024, 1024, 1024, 1024, 512, 256, 128, 128]
assert sum(SEGS) == L
MAXSEG = max(SEGS)

FP32 = mybir.dt.float32
F16 = mybir.dt.float16
_last_results = None


def build_nc() -> bass.Bass:
    # Bacc (not plain Bass): its compile() runs move_matmul_waits_to_ldweights
    # and generate_event_semaphores, which split multi-sem waits to satisfy the
    # TRN2 per-instruction wait-slot limits walrus enforces.
    nc = bacc.Bacc()

    xT = nc.dram_tensor("xT", [D, L], F16, kind="ExternalInput")
    wgT = nc.dram_tensor("wgT", [D, EH], F16, kind="ExternalInput")
    wcT = nc.dram_tensor("wcT", [D, EH], F16, kind="ExternalInput")
    # biases packed [128, 4]: cols 0..1 = bg per e-tile, 2..3 = bc per e-tile
    bias = nc.dram_tensor("bias", [128, 2 * NET], FP32, kind="ExternalInput")
    h = nc.dram_tensor("h", [EH, L], F16, kind="ExternalOutput")

    op = mybir.AluOpType
    act = mybir.ActivationFunctionType

    with TileContext(nc) as tc:
        with (
            tc.tile_pool(name="consts", bufs=1) as consts,
            tc.tile_pool(name="xpool", bufs=4) as xpool,
            tc.tile_pool(name="work", bufs=3) as work,
            tc.tile_pool(name="hpool", bufs=2) as hpool,
            tc.tile_pool(name="psum", bufs=2, space="PSUM") as psum,
        ):
            # First matmul needs wg + x seg 0: load them CONCURRENTLY on two
            # HWDGE queues (wg on sync, x0 on the scalar queue — the Act
            # engine is idle this early) so PE start latency is max() not
            # sum(). Later x segs ride the sync queue; biases ride the SWDGE
            # (gpsimd) queue.
            wg_sb = consts.tile([128, NDC, EH], F16)
            wc_sb = consts.tile([128, NDC, EH], F16)
            nc.sync.dma_start(wg_sb, wgT.rearrange("(c p) e -> p c e", p=128))
            x0_sb = xpool.tile([128, NDC, MAXSEG], F16, tag="x", name="x_0")[
                :, :, : SEGS[0]
            ]
            nc.scalar.dma_start(x0_sb, xT[:, 0 : SEGS[0]].rearrange("(c p) l -> p c l", p=128))
            nc.sync.dma_start(wc_sb, wcT.rearrange("(c p) e -> p c e", p=128))

            bias_sb = consts.tile([128, 2 * NET], FP32)
            bgn_sb = consts.tile([128, NET], FP32)
            nc.gpsimd.dma_start(bias_sb, bias[:])
            nc.scalar.mul(bgn_sb, bias_sb[:, 0:NET], -1.0)
            bc_sb = bias_sb[:, NET : 2 * NET]

            carry = [None] * NET  # [128, 1] AP of the previous h column

            l0 = 0
            for t, lt in enumerate(SEGS):
                nbk = (lt + NSUB - 1) // NSUB  # PSUM banks this segment
                if t == 0:
                    x_sb = x0_sb
                else:
                    x_sb = xpool.tile([128, NDC, MAXSEG], F16, tag="x", name=f"x_{t}")[
                        :, :, :lt
                    ]
                    nc.sync.dma_start(
                        x_sb, xT[:, l0 : l0 + lt].rearrange("(c p) l -> p c l", p=128)
                    )
                a_t = work.tile([128, NET, MAXSEG], F16, tag="a", name=f"a_{t}")
                c_t = work.tile([128, NET, MAXSEG], F16, tag="c", name=f"c_{t}")
                bn_t = work.tile([128, NET, MAXSEG], F16, tag="b", name=f"b_{t}")
                h_t = hpool.tile([128, NET, MAXSEG], F16, tag="h", name=f"h_{t}")
                for et in range(NET):
                    esl = slice(et * 128, (et + 1) * 128)
                    for n in range(nbk):
                        w = min(NSUB, lt - n * NSUB)
                        nsl = slice(n * NSUB, n * NSUB + w)
                        # One 2-bank PSUM tile per (et, 512-token chunk):
                        # [*, 0, :] = z_g, [*, 1, :] = z_c. bufs=2 per et tag
                        # double-buffers chunks so next-segment matmuls never
                        # convoy behind this segment's activations.
                        pz = psum.tile(
                            [128, 2, NSUB], FP32, tag=f"pz{et}", name=f"pz{et}_{t}_{n}"
                        )
                        for proj in range(2):
                            wsrc = wg_sb if proj == 0 else wc_sb
                            for dc in range(NDC):
                                nc.tensor.matmul(
                                    pz[:, proj, :w],
                                    wsrc[:, dc, esl],
                                    x_sb[:, dc, nsl],
                                    start=(dc == 0),
                                    stop=(dc == NDC - 1),
                                )
                        # a = sigmoid(-(z_g + bg)) = 1 - g ; c = tanh(z_c + bc)
                        nc.scalar.activation(
                            a_t[:, et, nsl], pz[:, 0, :w], act.Sigmoid,
                            bias=bgn_sb[:, et : et + 1], scale=-1.0,
                        )
                        nc.scalar.activation(
                            c_t[:, et, nsl], pz[:, 1, :w], act.Tanh,
                            bias=bc_sb[:, et : et + 1], scale=1.0,
                        )
                    # bneg = (a - 1) * c = -g * c  (one DVE op, full segment)
                    nc.vector.scalar_tensor_tensor(
                        bn_t[:, et, :lt], a_t[:, et, :lt], 1.0, c_t[:, et, :lt],
                        op.subtract, op.mult,
                    )
                    # h = a * h_prev - bneg  (fp32 state in HW, fp16 storage)
                    init = 0.0 if carry[et] is None else carry[et]
                    nc.vector.tensor_tensor_scan(
                        h_t[:, et, :lt], a_t[:, et, :lt], bn_t[:, et, :lt],
                        init, op.mult, op.subtract,
                    )
                    carry[et] = h_t[:, et, lt - 1 : lt]
                # h writes on the SWDGE queue: keeps the sync HWDGE queue a
                # pure x-feed. One DMA covers both e-tiles.
                nc.gpsimd.dma_start(
                    h[:, l0 : l0 + lt].rearrange("(e p) l -> p e l", p=128),
                    h_t[:, :, :lt],
                )
                l0 += lt
    return nc


def _in_maps(x, Wg, bg, Wc, bc):
    maps = []
    xT = {}
    for c in range(NCORES):
        b, eh = c // 2, c % 2
        e0 = eh * EH
        if b not in xT:
            xT[b] = np.ascontiguousarray(x[b].T.astype(np.float16))
        bias_pack = np.concatenate(
            [
                bg[e0 : e0 + EH].reshape(NET, 128).T,
                bc[e0 : e0 + EH].reshape(NET, 128).T,
            ],
            axis=1,
        )
        maps.append(
            {
                "xT": xT[b],
                "wgT": np.ascontiguousarray(Wg[e0 : e0 + EH].T.astype(np.float16)),
                "wcT": np.ascontiguousarray(Wc[e0 : e0 + EH].T.astype(np.float16)),
                "bias": np.ascontiguousarray(bias_pack.astype(np.float32)),
            }
        )
    return maps


def kernel(x, Wg, bg, Wc, bc):
    global _last_results
    x = np.asarray(x, dtype=np.float32)
    Wg = np.asarray(Wg, dtype=np.float32)
    bg = np.asarray(bg, dtype=np.float32)
    Wc = np.asarray(Wc, dtype=np.float32)
    bc = np.asarray(bc, dtype=np.float32)

    nc = build_nc()
    if not nc.is_finalized():
        nc.finalize()
    res = run_bass_kernel_spmd(
        nc,
        _in_maps(x, Wg, bg, Wc, bc),
        list(range(NCORES)),
        tmpdir=os.environ.get("KERNEL_TMPDIR"),
    )
    _last_results = res

    out = np.empty((B, L, D), dtype=np.float32)
    for b in range(B):
        hb = np.concatenate(
            [res.results[2 * b]["h"], res.results[2 * b + 1]["h"]], axis=0
        ).astype(np.float32)
        out[b] = hb.T
    return out
